# revision 14
# baseline (speedup 1.0000x reference)
"""Single-head attention on 8 trn2 NeuronCores.

Sharding: data-parallel over batch (B=8 -> one batch element per core, no
collectives). Host-side prep per core: transpose q/k/v to [E, S] and cast to
bf16 (half the DMA bytes, full PE rate), pre-pack the projection weights into
partition-major [128, 6*64] layout (128 DMA descriptors instead of 768), and
fold key_mask into a per-key log-bias consumed by the fused exp activation.
The output is produced transposed [H, S] (64 fat DMA descriptors instead of
2048 thin ones) and un-transposed on the host.

Per-core pipeline (S=2048, E=768, H=64), emission-ordered so the ACT exp
stream (the pacing engine, ~1.3us per [128,1024] tile) starts after only
~2.3MB of DMA and never starves:
  kT[64,S]    = Wk.T @ keyT    first column-quarter, then the rest
  qT0[64,1024]= Wq.T @ queryT  tile 0
  e0[c][128,1024] = exp((kT_c.T @ qT0)/8 + log_km_c)   c = 0..15
  qT1, e1[0..3]
  vT = Wv.T @ valueT -> PE-transpose -> v_aug[128,65] (65th col = 1.0 so
      the PV matmul also produces the softmax denominator)
  pairs: e1[c] alongside o0[65,1024] += v_aug.T @ e0    (PE interleaved)
  o1 accumulation, then per tile: recip(denominator row) -> gpsimd
  partition-broadcast -> DVE multiply -> DMA outT tile.

PSUM budget (8 banks): scores 2x2 banks, o-accumulator 1x2 banks, 2x1 bank
for short-lived projection/transpose tiles.

Softmax max-subtraction is skipped: scores ~ N(0,1) here (|s| < ~7),
far below f32 exp overflow.
"""

import sys

import numpy as np

for _p in ("/opt/trn_rl_repo",):
    if _p not in sys.path:
        sys.path.insert(0, _p)

from contextlib import ExitStack

import ml_dtypes
import concourse.bass as bass  # noqa: F401  (engine handles live on nc)
import concourse.tile as tile
from concourse import bacc, mybir
from concourse.bass_utils import run_bass_kernel_spmd
from concourse.masks import make_identity

B, S, E, H = 8, 2048, 768, 64
EC = E // 128            # 6 embedding chunks
SQT = 1024               # query-tile size
N_SQ = S // SQT          # 2
N_SK = S // 128          # 16 key chunks
KQ = 512                 # kT column-quarter width
F32 = mybir.dt.float32
BF16 = mybir.dt.bfloat16
EXP = mybir.ActivationFunctionType.Exp
BF = ml_dtypes.bfloat16

_built = None


def _build():
    nc = bacc.Bacc(
        "TRN2",
        target_bir_lowering=False,
        debug=False,
        enable_asserts=False,
        num_devices=8,
    )
    qT_in = nc.dram_tensor("qT", [E, S], BF16, kind="ExternalInput").ap()
    kT_in = nc.dram_tensor("kT", [E, S], BF16, kind="ExternalInput").ap()
    vT_in = nc.dram_tensor("vT", [E, S], BF16, kind="ExternalInput").ap()
    wq_in = nc.dram_tensor("wq", [128, EC * H], BF16, kind="ExternalInput").ap()
    wk_in = nc.dram_tensor("wk", [128, EC * H], BF16, kind="ExternalInput").ap()
    wv_in = nc.dram_tensor("wv", [128, EC * H], BF16, kind="ExternalInput").ap()
    bq_in = nc.dram_tensor("bq", [H], F32, kind="ExternalInput").ap()
    bk_in = nc.dram_tensor("bk", [H], F32, kind="ExternalInput").ap()
    bv_in = nc.dram_tensor("bv", [H], F32, kind="ExternalInput").ap()
    lkm_in = nc.dram_tensor("lkm", [128, N_SK], F32, kind="ExternalInput").ap()
    out = nc.dram_tensor("outT", [H, S], F32, kind="ExternalOutput").ap()

    with tile.TileContext(nc) as tc, ExitStack() as ctx:
        consts = ctx.enter_context(tc.tile_pool(name="consts", bufs=1))
        persist = ctx.enter_context(tc.tile_pool(name="persist", bufs=1))
        kslices = ctx.enter_context(tc.tile_pool(name="kslices", bufs=6))
        krest = ctx.enter_context(tc.tile_pool(name="krest", bufs=6))
        vchunks = ctx.enter_context(tc.tile_pool(name="vchunks", bufs=12))
        qchunks = ctx.enter_context(tc.tile_pool(name="qchunks", bufs=12))
        qtp = ctx.enter_context(tc.tile_pool(name="qtp", bufs=2))
        epool = ctx.enter_context(tc.tile_pool(name="epool", bufs=24))
        opool = ctx.enter_context(tc.tile_pool(name="opool", bufs=2))
        fpool = ctx.enter_context(tc.tile_pool(name="fpool", bufs=2))
        spsum = ctx.enter_context(tc.tile_pool(name="spsum", bufs=2, space="PSUM"))
        opsum = ctx.enter_context(tc.tile_pool(name="opsum", bufs=1, space="PSUM"))
        mpsum = ctx.enter_context(tc.tile_pool(name="mpsum", bufs=2, space="PSUM"))

        def dma_split(out_tile, in_ap, ways):
            # Split a [128, N] transfer into partition groups so several DMA
            # queues work the transfer in parallel (per-queue latency is the
            # head-of-pipeline gate, not aggregate bandwidth).
            rows = out_tile.shape[0]
            step = rows // ways
            for w in range(ways):
                r0 = w * step
                r1 = rows if w == ways - 1 else r0 + step
                nc.sync.dma_start(out=out_tile[r0:r1], in_=in_ap[r0:r1])

        def mtile(shape, dtype):
            # Two rotating single-bank PSUM slots for short-lived projection
            # and transpose tiles.
            t = mpsum.tile(shape, dtype, tag="m")
            return t

        ident_bf = consts.tile([128, 128], BF16, tag="ident_bf")
        make_identity(nc, ident_bf[:])
        lkm_sb = consts.tile([128, N_SK], F32)
        nc.sync.dma_start(out=lkm_sb[:], in_=lkm_in[:])

        w_sb = {}
        b_sb = {}
        for name, w_ap, b_ap in (
            ("q", wq_in, bq_in),
            ("k", wk_in, bk_in),
            ("v", wv_in, bv_in),
        ):
            w = consts.tile([128, EC, H], BF16, tag=f"w{name}")
            nc.sync.dma_start(out=w[:], in_=w_ap.rearrange("p (c h) -> p c h", c=EC))
            bias = consts.tile([H, 1], F32, tag=f"b{name}")
            nc.sync.dma_start(out=bias[:], in_=b_ap.rearrange("(h one) -> h one", one=1))
            w_sb[name] = w
            b_sb[name] = bias

        kT_sb = persist.tile([H, S], BF16, tag="kT")
        vT_sb = persist.tile([H, S], BF16, tag="vT")

        def project(ps, wname, rhs_slices):
            for c in range(EC):
                nc.tensor.matmul(
                    ps[:], w_sb[wname][:, c, :], rhs_slices[c],
                    start=(c == 0), stop=(c == EC - 1),
                )

        # ---- K: first column-quarter via narrow slices, rest as one fat DMA.
        ksl = []
        for c in range(EC):
            ks = kslices.tile([128, KQ], BF16, tag="kslice")
            dma_split(ks, kT_in[c * 128 : (c + 1) * 128, 0:KQ], 2)
            ksl.append(ks)
        ps = mtile([H, KQ], F32)
        project(ps, "k", [ks[:] for ks in ksl])
        nc.vector.tensor_scalar_add(kT_sb[:, 0:KQ], ps[:], b_sb["k"][:])

        def q_tile(i):
            qch = []
            for c in range(EC):
                qc = qchunks.tile([128, SQT], BF16, tag="qchunk")
                dma_split(qc, qT_in[c * 128 : (c + 1) * 128, i * SQT : (i + 1) * SQT], 4)
                qch.append(qc)
            qt = qtp.tile([H, SQT], BF16, tag="qt")
            for h in range(SQT // 512):
                ps = mtile([H, 512], F32)
                project(ps, "q", [qc[:, h * 512 : (h + 1) * 512] for qc in qch])
                nc.vector.tensor_scalar_add(qt[:, h * 512 : (h + 1) * 512], ps[:], b_sb["q"][:])
            return qt

        qt0 = q_tile(0)

        kre = []
        for c in range(EC):
            kr = krest.tile([128, S - KQ], BF16, tag="krest")
            dma_split(kr, kT_in[c * 128 : (c + 1) * 128, KQ:S], 4)
            kre.append(kr)

        def k_quarter(q):
            c0 = q * KQ
            ps = mtile([H, KQ], F32)
            project(ps, "k", [kr[:, c0 - KQ : c0 - KQ + KQ] for kr in kre])
            nc.vector.tensor_scalar_add(kT_sb[:, c0 : c0 + KQ], ps[:], b_sb["k"][:])

        def score_exp(qt, c):
            sp = spsum.tile([128, SQT], F32, tag="sp")
            for h in range(SQT // 512):
                nc.tensor.matmul(
                    sp[:, h * 512 : (h + 1) * 512],
                    kT_sb[:, c * 128 : (c + 1) * 128],
                    qt[:, h * 512 : (h + 1) * 512],
                    start=True, stop=True,
                )
            e = epool.tile([128, SQT], BF16, tag="e")
            nc.scalar.activation(e[:], sp[:], EXP, bias=lkm_sb[:, c : c + 1], scale=0.125)
            return e

        # exp chunks interleaved with the kT quarter projections that feed
        # the next group, so the ACT stream never waits on the whole K DMA.
        e0 = []
        for q in range(4):
            for c in range(4 * q, 4 * q + 4):
                e0.append(score_exp(qt0, c))
            if q < 3:
                k_quarter(q + 1)
        qt1 = q_tile(1)
        e1 = [score_exp(qt1, c) for c in range(4)]

        # ---- V phase: halved DMAs; per column-quarter project + transpose.
        vch = []
        for c in range(EC):
            for hh in range(2):
                vc = vchunks.tile([128, SQT], BF16, tag="vchunk")
                dma_split(vc, vT_in[c * 128 : (c + 1) * 128, hh * SQT : (hh + 1) * SQT], 2)
                vch.append((c, hh, vc))
        vhalf = {(c, hh): vc for c, hh, vc in vch}
        vaug = []
        for t in range(N_SK):
            va = persist.tile([128, H + 1], BF16, tag=f"vaug{t}")
            vaug.append(va)

        def v_quarter(q):
            c0 = q * KQ
            hh = q // 2
            off = c0 - hh * SQT
            ps = mtile([H, KQ], F32)
            project(ps, "v", [vhalf[(c, hh)][:, off : off + KQ] for c in range(EC)])
            nc.vector.tensor_scalar_add(vT_sb[:, c0 : c0 + KQ], ps[:], b_sb["v"][:])
            for t in range(4 * q, 4 * q + 4):
                tpv = mtile([128, H], BF16)
                nc.tensor.transpose(tpv[:], vT_sb[:, t * 128 : (t + 1) * 128], ident_bf[:H, :H])
                nc.vector.memset(vaug[t][:, 0:1], 1.0)
                nc.vector.tensor_copy(vaug[t][:, 1 : H + 1], tpv[:])

        v_quarter(0)

        def pv(ops, c, e):
            for h in range(SQT // 512):
                nc.tensor.matmul(
                    ops[:, h * 512 : (h + 1) * 512],
                    vaug[c][:],
                    e[:, h * 512 : (h + 1) * 512],
                    start=(c == 0), stop=(c == N_SK - 1),
                )

        # ---- pairs: tile-1 scores/exp interleaved with tile-0 PV (lag 4).
        ops0 = opsum.tile([H + 1, SQT], F32, tag="ops")
        for c in range(4, N_SK):
            e1.append(score_exp(qt1, c))
            pv(ops0, c - 4, e0[c - 4])
            if c in (7, 11):
                v_quarter(c // 4)
        v_quarter(3)
        for c in range(N_SK - 4, N_SK):
            pv(ops0, c, e0[c])
        osb0 = opool.tile([H + 1, SQT], F32, tag="osb")
        nc.vector.tensor_copy(osb0[:], ops0[:])

        def finalize_half(i, h, osb_half):
            # osb_half: [H+1, 512] in SBUF, denominator on partition 0.
            rc = fpool.tile([1, 512], F32, tag="rc")
            nc.vector.reciprocal_approx_fast(rc[:], osb_half[0:1, :])
            rcb = fpool.tile([H + 1, 512], F32, tag="rcb")
            nc.gpsimd.partition_broadcast(rcb[:], rc[:], channels=H + 1)
            ot = fpool.tile([H + 1, 512], F32, tag="ot")
            nc.vector.tensor_mul(ot[:], osb_half[:], rcb[:])
            c0 = i * SQT + h * 512
            nc.sync.dma_start(out=out[:, c0 : c0 + 512], in_=ot[1 : H + 1, :])

        finalize_half(0, 0, osb0[:, 0:512])
        finalize_half(0, 1, osb0[:, 512:1024])

        # tile-1 PV by column halves: half 0 finalizes while PE runs half 1.
        ops1 = opsum.tile([H + 1, SQT], F32, tag="ops")
        for h in range(2):
            for c in range(N_SK):
                nc.tensor.matmul(
                    ops1[:, h * 512 : (h + 1) * 512],
                    vaug[c][:],
                    e1[c][:, h * 512 : (h + 1) * 512],
                    start=(c == 0), stop=(c == N_SK - 1),
                )
            osb1h = opool.tile([H + 1, 512], F32, tag="osb1h")
            nc.vector.tensor_copy(osb1h[:], ops1[:, h * 512 : (h + 1) * 512])
            finalize_half(1, h, osb1h[:])

    nc.compile()
    return nc


def _get_built():
    global _built
    if _built is None:
        _built = _build()
    return _built


def _in_maps(query, key, value, key_mask, Wq, bq, Wk, bk, Wv, bv):
    f32 = lambda a: np.asarray(a, dtype=np.float32)
    bf = lambda a: np.ascontiguousarray(np.asarray(a, dtype=np.float32).astype(BF))

    def packw(w):
        # [768, 64] -> partition-major [128, 6*64]
        w = np.asarray(w, dtype=np.float32).astype(BF)
        return np.ascontiguousarray(w.reshape(EC, 128, H).transpose(1, 0, 2).reshape(128, EC * H))

    Wq_b, Wk_b, Wv_b = packw(Wq), packw(Wk), packw(Wv)
    bq, bk, bv = f32(bq), f32(bk), f32(bv)
    maps = []
    for b in range(B):
        with np.errstate(divide="ignore"):
            lkm = np.log(f32(key_mask[b]))
        maps.append(
            {
                "qT": bf(np.asarray(query[b]).T),
                "kT": bf(np.asarray(key[b]).T),
                "vT": bf(np.asarray(value[b]).T),
                "wq": Wq_b,
                "wk": Wk_b,
                "wv": Wv_b,
                "bq": bq,
                "bk": bk,
                "bv": bv,
                "lkm": np.ascontiguousarray(lkm.reshape(N_SK, 128).T),
            }
        )
    return maps


def run(trace=False, **inputs):
    nc = _get_built()
    maps = _in_maps(
        inputs["query"],
        inputs["key"],
        inputs["value"],
        inputs["key_mask"],
        inputs["Wq"],
        inputs["bq"],
        inputs["Wk"],
        inputs["bk"],
        inputs["Wv"],
        inputs["bv"],
    )
    res = run_bass_kernel_spmd(nc, maps, core_ids=list(range(B)), trace=trace)
    full = np.stack(
        [np.ascontiguousarray(res.results[i]["outT"].T) for i in range(B)]
    ).astype(np.float32)
    return full, res


def kernel(**inputs):
    full, _ = run(trace=False, **inputs)
    return full


# revision 15
# speedup vs baseline: 1.2804x; 1.2804x over previous
"""Single-head attention on 8 trn2 NeuronCores.

Sharding: data-parallel over batch (B=8 -> one batch element per core, no
collectives). Host-side prep per core: transpose q/k/v to [E, S] and cast to
bf16 (half the DMA bytes, full PE rate), pre-pack the projection weights into
partition-major [128, 6*64] layout (128 DMA descriptors instead of 768), and
fold key_mask into a per-key log-bias consumed by the fused exp activation.
The output is produced transposed [H, S] (64 fat DMA descriptors instead of
2048 thin ones) and un-transposed on the host.

Per-core pipeline (S=2048, E=768, H=64), emission-ordered so the ACT exp
stream (the pacing engine, ~1.3us per [128,1024] tile) starts after only
~2.3MB of DMA and never starves:
  kT[64,S]    = Wk.T @ keyT    first column-quarter, then the rest
  qT0[64,1024]= Wq.T @ queryT  tile 0
  e0[c][128,1024] = exp((kT_c.T @ qT0)/8 + log_km_c)   c = 0..15
  qT1, e1[0..3]
  vT = Wv.T @ valueT -> PE-transpose -> v_aug[128,65] (65th col = 1.0 so
      the PV matmul also produces the softmax denominator)
  pairs: e1[c] alongside o0[65,1024] += v_aug.T @ e0    (PE interleaved)
  o1 accumulation, then per tile: recip(denominator row) -> gpsimd
  partition-broadcast -> DVE multiply -> DMA outT tile.

PSUM budget (8 banks): scores 2x2 banks, o-accumulator 1x2 banks, 2x1 bank
for short-lived projection/transpose tiles.

Softmax max-subtraction is skipped: scores ~ N(0,1) here (|s| < ~7),
far below f32 exp overflow.
"""

import sys

import numpy as np

for _p in ("/opt/trn_rl_repo",):
    if _p not in sys.path:
        sys.path.insert(0, _p)

from contextlib import ExitStack

import ml_dtypes
import concourse.bass as bass  # noqa: F401  (engine handles live on nc)
import concourse.tile as tile
from concourse import bacc, mybir
from concourse.bass_utils import run_bass_kernel_spmd
from concourse.masks import make_identity

B, S, E, H = 8, 2048, 768, 64
EC = E // 128            # 6 embedding chunks
SQT = 1024               # query-tile size
N_SQ = S // SQT          # 2
N_SK = S // 128          # 16 key chunks
KQ = 512                 # kT column-quarter width
F32 = mybir.dt.float32
BF16 = mybir.dt.bfloat16
EXP = mybir.ActivationFunctionType.Exp
BF = ml_dtypes.bfloat16

_built = None


def _build():
    nc = bacc.Bacc(
        "TRN2",
        target_bir_lowering=False,
        debug=False,
        enable_asserts=False,
        num_devices=8,
    )
    qT_in = nc.dram_tensor("qT", [E, S], BF16, kind="ExternalInput").ap()
    kT_in = nc.dram_tensor("kT", [E, S], BF16, kind="ExternalInput").ap()
    vT_in = nc.dram_tensor("vT", [E, S], BF16, kind="ExternalInput").ap()
    wq_in = nc.dram_tensor("wq", [128, EC * H], BF16, kind="ExternalInput").ap()
    wk_in = nc.dram_tensor("wk", [128, EC * H], BF16, kind="ExternalInput").ap()
    wv_in = nc.dram_tensor("wv", [128, EC * H], BF16, kind="ExternalInput").ap()
    bq_in = nc.dram_tensor("bq", [H], F32, kind="ExternalInput").ap()
    bk_in = nc.dram_tensor("bk", [H], F32, kind="ExternalInput").ap()
    bv_in = nc.dram_tensor("bv", [H], F32, kind="ExternalInput").ap()
    lkm_in = nc.dram_tensor("lkm", [128, N_SK], F32, kind="ExternalInput").ap()
    out = nc.dram_tensor("outT", [H, S], F32, kind="ExternalOutput").ap()

    with tile.TileContext(nc) as tc, ExitStack() as ctx:
        consts = ctx.enter_context(tc.tile_pool(name="consts", bufs=1))
        persist = ctx.enter_context(tc.tile_pool(name="persist", bufs=1))
        kslices = ctx.enter_context(tc.tile_pool(name="kslices", bufs=6))
        krest = ctx.enter_context(tc.tile_pool(name="krest", bufs=6))
        vchunks = ctx.enter_context(tc.tile_pool(name="vchunks", bufs=12))
        qchunks = ctx.enter_context(tc.tile_pool(name="qchunks", bufs=12))
        qtp = ctx.enter_context(tc.tile_pool(name="qtp", bufs=2))
        epool = ctx.enter_context(tc.tile_pool(name="epool", bufs=24))
        opool = ctx.enter_context(tc.tile_pool(name="opool", bufs=2))
        fpool = ctx.enter_context(tc.tile_pool(name="fpool", bufs=2))
        spsum = ctx.enter_context(tc.tile_pool(name="spsum", bufs=2, space="PSUM"))
        opsum = ctx.enter_context(tc.tile_pool(name="opsum", bufs=1, space="PSUM"))
        mpsum = ctx.enter_context(tc.tile_pool(name="mpsum", bufs=2, space="PSUM"))

        def dma_split(out_tile, in_ap, ways):
            # Split a [128, N] transfer into partition groups so several DMA
            # queues work the transfer in parallel (per-queue latency is the
            # head-of-pipeline gate, not aggregate bandwidth).
            rows = out_tile.shape[0]
            step = rows // ways
            for w in range(ways):
                r0 = w * step
                r1 = rows if w == ways - 1 else r0 + step
                nc.sync.dma_start(out=out_tile[r0:r1], in_=in_ap[r0:r1])

        def mtile(shape, dtype):
            # Two rotating single-bank PSUM slots for short-lived projection
            # and transpose tiles.
            t = mpsum.tile(shape, dtype, tag="m")
            return t

        ident_bf = consts.tile([128, 128], BF16, tag="ident_bf")
        make_identity(nc, ident_bf[:])
        lkm_sb = consts.tile([128, N_SK], F32)
        nc.sync.dma_start(out=lkm_sb[:], in_=lkm_in[:])

        w_sb = {}
        b_sb = {}
        for name, w_ap, b_ap in (
            ("q", wq_in, bq_in),
            ("k", wk_in, bk_in),
            ("v", wv_in, bv_in),
        ):
            w = consts.tile([128, EC, H], BF16, tag=f"w{name}")
            nc.sync.dma_start(out=w[:], in_=w_ap.rearrange("p (c h) -> p c h", c=EC))
            bias = consts.tile([H, 1], F32, tag=f"b{name}")
            nc.sync.dma_start(out=bias[:], in_=b_ap.rearrange("(h one) -> h one", one=1))
            w_sb[name] = w
            b_sb[name] = bias

        kT_sb = persist.tile([H, S], BF16, tag="kT")
        vT_sb = persist.tile([H, S], BF16, tag="vT")

        def project(ps, wname, rhs_slices):
            for c in range(EC):
                nc.tensor.matmul(
                    ps[:], w_sb[wname][:, c, :], rhs_slices[c],
                    start=(c == 0), stop=(c == EC - 1),
                )

        # ---- K: first column-quarter via narrow slices, rest as one fat DMA.
        ksl = []
        for c in range(EC):
            ks = kslices.tile([128, KQ], BF16, tag="kslice")
            nc.sync.dma_start(out=ks[:], in_=kT_in[c * 128 : (c + 1) * 128, 0:KQ])
            ksl.append(ks)
        ps = mtile([H, KQ], F32)
        project(ps, "k", [ks[:] for ks in ksl])
        nc.vector.tensor_scalar_add(kT_sb[:, 0:KQ], ps[:], b_sb["k"][:])

        def q_tile(i):
            qch = []
            for c in range(EC):
                qc = qchunks.tile([128, SQT], BF16, tag="qchunk")
                nc.sync.dma_start(out=qc[:], in_=qT_in[c * 128 : (c + 1) * 128, i * SQT : (i + 1) * SQT])
                qch.append(qc)
            qt = qtp.tile([H, SQT], BF16, tag="qt")
            for h in range(SQT // 512):
                ps = mtile([H, 512], F32)
                project(ps, "q", [qc[:, h * 512 : (h + 1) * 512] for qc in qch])
                nc.vector.tensor_scalar_add(qt[:, h * 512 : (h + 1) * 512], ps[:], b_sb["q"][:])
            return qt

        qt0 = q_tile(0)

        kre = []
        for c in range(EC):
            kr = krest.tile([128, S - KQ], BF16, tag="krest")
            nc.sync.dma_start(out=kr[:], in_=kT_in[c * 128 : (c + 1) * 128, KQ:S])
            kre.append(kr)

        def k_quarter(q):
            c0 = q * KQ
            ps = mtile([H, KQ], F32)
            project(ps, "k", [kr[:, c0 - KQ : c0 - KQ + KQ] for kr in kre])
            nc.vector.tensor_scalar_add(kT_sb[:, c0 : c0 + KQ], ps[:], b_sb["k"][:])

        def score_exp(qt, c):
            sp = spsum.tile([128, SQT], F32, tag="sp")
            for h in range(SQT // 512):
                nc.tensor.matmul(
                    sp[:, h * 512 : (h + 1) * 512],
                    kT_sb[:, c * 128 : (c + 1) * 128],
                    qt[:, h * 512 : (h + 1) * 512],
                    start=True, stop=True,
                )
            e = epool.tile([128, SQT], BF16, tag="e")
            nc.scalar.activation(e[:], sp[:], EXP, bias=lkm_sb[:, c : c + 1], scale=0.125)
            return e

        # exp chunks interleaved with the kT quarter projections that feed
        # the next group, so the ACT stream never waits on the whole K DMA.
        e0 = []
        for q in range(4):
            for c in range(4 * q, 4 * q + 4):
                e0.append(score_exp(qt0, c))
            if q < 3:
                k_quarter(q + 1)
        qt1 = q_tile(1)
        e1 = [score_exp(qt1, c) for c in range(4)]

        # ---- V phase: halved DMAs; per column-quarter project + transpose.
        vch = []
        for c in range(EC):
            for hh in range(2):
                vc = vchunks.tile([128, SQT], BF16, tag="vchunk")
                nc.sync.dma_start(out=vc[:], in_=vT_in[c * 128 : (c + 1) * 128, hh * SQT : (hh + 1) * SQT])
                vch.append((c, hh, vc))
        vhalf = {(c, hh): vc for c, hh, vc in vch}
        vaug = []
        for t in range(N_SK):
            va = persist.tile([128, H + 1], BF16, tag=f"vaug{t}")
            vaug.append(va)

        def v_quarter(q):
            c0 = q * KQ
            hh = q // 2
            off = c0 - hh * SQT
            ps = mtile([H, KQ], F32)
            project(ps, "v", [vhalf[(c, hh)][:, off : off + KQ] for c in range(EC)])
            nc.vector.tensor_scalar_add(vT_sb[:, c0 : c0 + KQ], ps[:], b_sb["v"][:])
            for t in range(4 * q, 4 * q + 4):
                tpv = mtile([128, H], BF16)
                nc.tensor.transpose(tpv[:], vT_sb[:, t * 128 : (t + 1) * 128], ident_bf[:H, :H])
                nc.vector.memset(vaug[t][:, 0:1], 1.0)
                nc.vector.tensor_copy(vaug[t][:, 1 : H + 1], tpv[:])

        v_quarter(0)

        def pv(ops, c, e):
            for h in range(SQT // 512):
                nc.tensor.matmul(
                    ops[:, h * 512 : (h + 1) * 512],
                    vaug[c][:],
                    e[:, h * 512 : (h + 1) * 512],
                    start=(c == 0), stop=(c == N_SK - 1),
                )

        # ---- pairs: tile-1 scores/exp interleaved with tile-0 PV (lag 4).
        ops0 = opsum.tile([H + 1, SQT], F32, tag="ops")
        for c in range(4, N_SK):
            e1.append(score_exp(qt1, c))
            pv(ops0, c - 4, e0[c - 4])
            if c in (7, 11):
                v_quarter(c // 4)
        v_quarter(3)
        for c in range(N_SK - 4, N_SK):
            pv(ops0, c, e0[c])
        osb0 = opool.tile([H + 1, SQT], F32, tag="osb")
        nc.vector.tensor_copy(osb0[:], ops0[:])

        def finalize_half(i, h, osb_half):
            # osb_half: [H+1, 512] in SBUF, denominator on partition 0.
            rc = fpool.tile([1, 512], F32, tag="rc")
            nc.vector.reciprocal_approx_fast(rc[:], osb_half[0:1, :])
            rcb = fpool.tile([H + 1, 512], F32, tag="rcb")
            nc.gpsimd.partition_broadcast(rcb[:], rc[:], channels=H + 1)
            ot = fpool.tile([H + 1, 512], F32, tag="ot")
            nc.vector.tensor_mul(ot[:], osb_half[:], rcb[:])
            c0 = i * SQT + h * 512
            nc.sync.dma_start(out=out[:, c0 : c0 + 512], in_=ot[1 : H + 1, :])

        finalize_half(0, 0, osb0[:, 0:512])
        finalize_half(0, 1, osb0[:, 512:1024])

        # tile-1 PV by column halves: half 0 finalizes while PE runs half 1.
        ops1 = opsum.tile([H + 1, SQT], F32, tag="ops")
        for h in range(2):
            for c in range(N_SK):
                nc.tensor.matmul(
                    ops1[:, h * 512 : (h + 1) * 512],
                    vaug[c][:],
                    e1[c][:, h * 512 : (h + 1) * 512],
                    start=(c == 0), stop=(c == N_SK - 1),
                )
            osb1h = opool.tile([H + 1, 512], F32, tag="osb1h")
            nc.vector.tensor_copy(osb1h[:], ops1[:, h * 512 : (h + 1) * 512])
            finalize_half(1, h, osb1h[:])

    nc.compile()
    return nc


def _get_built():
    global _built
    if _built is None:
        _built = _build()
    return _built


def _in_maps(query, key, value, key_mask, Wq, bq, Wk, bk, Wv, bv):
    f32 = lambda a: np.asarray(a, dtype=np.float32)
    bf = lambda a: np.ascontiguousarray(np.asarray(a, dtype=np.float32).astype(BF))

    def packw(w):
        # [768, 64] -> partition-major [128, 6*64]
        w = np.asarray(w, dtype=np.float32).astype(BF)
        return np.ascontiguousarray(w.reshape(EC, 128, H).transpose(1, 0, 2).reshape(128, EC * H))

    Wq_b, Wk_b, Wv_b = packw(Wq), packw(Wk), packw(Wv)
    bq, bk, bv = f32(bq), f32(bk), f32(bv)
    maps = []
    for b in range(B):
        with np.errstate(divide="ignore"):
            lkm = np.log(f32(key_mask[b]))
        maps.append(
            {
                "qT": bf(np.asarray(query[b]).T),
                "kT": bf(np.asarray(key[b]).T),
                "vT": bf(np.asarray(value[b]).T),
                "wq": Wq_b,
                "wk": Wk_b,
                "wv": Wv_b,
                "bq": bq,
                "bk": bk,
                "bv": bv,
                "lkm": np.ascontiguousarray(lkm.reshape(N_SK, 128).T),
            }
        )
    return maps


def run(trace=False, **inputs):
    nc = _get_built()
    maps = _in_maps(
        inputs["query"],
        inputs["key"],
        inputs["value"],
        inputs["key_mask"],
        inputs["Wq"],
        inputs["bq"],
        inputs["Wk"],
        inputs["bk"],
        inputs["Wv"],
        inputs["bv"],
    )
    res = run_bass_kernel_spmd(nc, maps, core_ids=list(range(B)), trace=trace)
    full = np.stack(
        [np.ascontiguousarray(res.results[i]["outT"].T) for i in range(B)]
    ).astype(np.float32)
    return full, res


def kernel(**inputs):
    full, _ = run(trace=False, **inputs)
    return full
